# revision 8
# baseline (speedup 1.0000x reference)
"""HONU (order-2, L=64) forward as a per-row quadratic form on 8 trn2 cores.

Reference computes out[i] = sum_{j<=k} W[p(j,k)] * x[i,j] * x[i,k] + b,
i.e. out[i] = x_i^T A x_i + b with A the 64x64 upper-triangular matrix
scattered from W.  We shard the batch across 8 cores (pure data parallel)
and evaluate the quadratic form with TensorE matmuls:

  per 256-row pair-tile:  xp (128p, 128f) holds two 128-row blocks side by
  side; TensorE-transpose gives xT with the two blocks' features stacked on
  partitions (2*64); a single matmul with blockdiag(A, A) yields yT = (x@A)^T
  for both blocks; z = xT*yT elementwise; a matmul with the (128,2) block-ones
  matrix E reduces partitions, giving the two blocks' outputs; ScalarE adds b.
"""

import math
from contextlib import ExitStack
from itertools import combinations_with_replacement

import numpy as np

import concourse.bacc as bacc
import concourse.bass as bass
import concourse.tile as tile
from concourse import mybir
from concourse.bass_utils import run_bass_kernel_spmd

L = 64
ORDER = 2
B = 16384
N_CORES = 8
SHARD = B // N_CORES  # 2048
PAIRS = SHARD // 256  # 8
NUM_W = math.comb(L + 1 + ORDER - 1, ORDER)  # 2145 (only first 2080 used)

IDX = np.array(list(combinations_with_replacement(range(L), ORDER)), dtype=np.int32)

F32 = mybir.dt.float32

_program_cache = {}


def _build_program(compile: bool = True) -> bass.Bass:
    nc = bacc.Bacc()

    x_in = nc.declare_dram_parameter("x", [SHARD, L], F32, isOutput=False)
    a2_in = nc.declare_dram_parameter("a2", [128, 128], F32, isOutput=False)
    eye_in = nc.declare_dram_parameter("eye", [128, 128], F32, isOutput=False)
    ew_in = nc.declare_dram_parameter("ew", [128, 2], F32, isOutput=False)
    bv_in = nc.declare_dram_parameter("bv", [2, 1], F32, isOutput=False)
    out_t = nc.declare_dram_parameter("out", [SHARD, 1], F32, isOutput=True)

    # row = two*1024 + q*128 + p  ->  [q][p, two, m]
    xv = x_in[:, :].rearrange("(two q p) m -> q p two m", two=2, p=128)
    # out rows: partition 0 -> rows 0..1023, partition 1 -> rows 1024..2047
    out_v = out_t[:, :].rearrange("(two f) one -> two (f one)", two=2)

    with ExitStack() as ctx:
        tc = ctx.enter_context(tile.TileContext(nc))
        consts = ctx.enter_context(tc.tile_pool(name="consts", bufs=1))
        xin_pool = ctx.enter_context(tc.tile_pool(name="xin", bufs=PAIRS))
        xt_pool = ctx.enter_context(tc.tile_pool(name="xt", bufs=PAIRS))
        z_pool = ctx.enter_context(tc.tile_pool(name="z", bufs=PAIRS))
        out_pool = ctx.enter_context(tc.tile_pool(name="outp", bufs=1))
        ps_xt = ctx.enter_context(tc.tile_pool(name="ps_xt", bufs=2, space="PSUM"))
        ps_yt = ctx.enter_context(tc.tile_pool(name="ps_yt", bufs=2, space="PSUM"))
        ps_o = ctx.enter_context(tc.tile_pool(name="ps_o", bufs=2, space="PSUM"))

        a2 = consts.tile([128, 128], F32)
        nc.sync.dma_start(out=a2[:], in_=a2_in[:, :])
        eye = consts.tile([128, 128], F32)
        nc.sync.dma_start(out=eye[:], in_=eye_in[:, :])
        ew = consts.tile([128, 2], F32)
        nc.sync.dma_start(out=ew[:], in_=ew_in[:, :])
        bv = consts.tile([2, 1], F32)
        nc.sync.dma_start(out=bv[:], in_=bv_in[:, :])

        out_sb = out_pool.tile([2, SHARD // 2], F32)

        # PE warmup matmuls: touch each constant once on the PE so that the
        # real matmuls in the loop never need more than one sync wait each
        # (walrus rejects Matmult instructions with >1 wait: "Too many sync
        # wait commands").
        warm_ps = ctx.enter_context(tc.tile_pool(name="warm", bufs=1, space="PSUM"))
        w1 = warm_ps.tile([128, 3], F32)
        nc.tensor.matmul(w1[:, 0:1], lhsT=eye[:], rhs=eye[:, 0:1], start=True, stop=True)
        nc.tensor.matmul(w1[:2, 1:2], lhsT=ew[:], rhs=eye[:, 0:1], start=True, stop=True)
        nc.tensor.matmul(w1[:, 2:3], lhsT=a2[:], rhs=eye[:, 0:1], start=True, stop=True)

        for q in range(PAIRS):
            xp = xin_pool.tile([128, 2, L], F32)
            nc.sync.dma_start(out=xp[:], in_=xv[q])
            xp2 = xp[:].rearrange("p two m -> p (two m)")

            pxt = ps_xt.tile([128, 128], F32)
            nc.tensor.transpose(pxt[:], xp2, eye[:])

            xt = xt_pool.tile([128, 128], F32)
            nc.vector.tensor_copy(xt[:], pxt[:])

            pyt = ps_yt.tile([128, 128], F32)
            nc.tensor.matmul(pyt[:], lhsT=a2[:], rhs=xt[:], start=True, stop=True)

            z = z_pool.tile([128, 128], F32)
            nc.vector.tensor_mul(z[:], xt[:], pyt[:])

            po = ps_o.tile([2, 128], F32)
            nc.tensor.matmul(po[:], lhsT=ew[:], rhs=z[:], start=True, stop=True)

            nc.vector.tensor_scalar_add(
                out_sb[:, q * 128 : (q + 1) * 128], po[:], bv[:]
            )

        nc.sync.dma_start(out=out_v, in_=out_sb[:])

    if compile:
        # Runs move_matmul_waits_to_ldweights + generate_event_semaphores,
        # which legalize instructions carrying more than one sync wait
        # (walrus rejects those with "Too many sync wait commands").
        nc.compile()
    return nc


def _get_program() -> bass.Bass:
    if "nc" not in _program_cache:
        _program_cache["nc"] = _build_program()
    return _program_cache["nc"]


def _host_constants(W: np.ndarray, b: np.ndarray):
    A = np.zeros((L, L), dtype=np.float32)
    A[IDX[:, 0], IDX[:, 1]] = W[: IDX.shape[0]].astype(np.float32)
    A2 = np.zeros((128, 128), dtype=np.float32)
    A2[:64, :64] = A
    A2[64:, 64:] = A
    eye = np.eye(128, dtype=np.float32)
    ew = np.zeros((128, 2), dtype=np.float32)
    ew[:64, 0] = 1.0
    ew[64:, 1] = 1.0
    bv = np.full((2, 1), np.float32(b.reshape(-1)[0]), dtype=np.float32)
    return A2, eye, ew, bv


def _run(x, W, b, trace=False):
    x = np.ascontiguousarray(np.asarray(x, dtype=np.float32))
    W = np.asarray(W, dtype=np.float32)
    b = np.asarray(b, dtype=np.float32)
    assert x.shape == (B, L), x.shape

    A2, eye, ew, bv = _host_constants(W, b)
    nc = _get_program()
    in_maps = [
        {
            "x": x[c * SHARD : (c + 1) * SHARD],
            "a2": A2,
            "eye": eye,
            "ew": ew,
            "bv": bv,
        }
        for c in range(N_CORES)
    ]
    res = run_bass_kernel_spmd(nc, in_maps, core_ids=list(range(N_CORES)), trace=trace)
    out = np.concatenate([res.results[c]["out"] for c in range(N_CORES)], axis=0)
    return out, res


def kernel(x, W, b):
    out, _ = _run(x, W, b)
    return out
